# revision 6
# baseline (speedup 1.0000x reference)
"""Trainium2 Bass kernel for nn_BertForOrdering pointer-network ordering loss.

Strategy: data-parallel over batch B=16 across 8 NeuronCores (2 batch
elements per core, same SPMD program).  Per batch element the device
computes q = dec@Wq+bq, k = sen@Wk+bk (TensorE), the pointer scores
scores[t,j] = sum_h wt[h]*tanh(q[t,h]+k[j,h]) (VectorE broadcast-adds +
ScalarE tanh + TensorE matvec reduction), and the masked row/col softmax
statistics (max, sum-exp) plus the score gathered at target[t].  The host
shards the inputs, builds the {0,-1e9} masks from target/tgt_len, and
combines the per-batch statistics into the final scalar loss (the final
log/gather/mean "all-reduce" step).
"""

import numpy as np

import bass_rust
import concourse.bass as bass
import concourse.tile as tile
from concourse import mybir
from concourse.vector_clock import ScopedClock
from concourse.bass_utils import run_bass_kernel_spmd
from concourse.masks import make_identity

B, N, H = 16, 128, 768
NCORES = 8
NB = B // NCORES          # batch elements per core
HC = H // 128             # h chunks of 128
TG = 32                   # t-group size for the big stage
NTG = N // TG
NEG = np.float32(-1e9)

F32 = mybir.dt.float32
BF16 = mybir.dt.bfloat16


class SafeTileContext(tile.TileContext):
    """Splits the tail-drain's sem waits into 1-wait carrier instructions:
    the b16 walrus caps sync-wait commands per CTRL instruction at 1."""

    MAXW = 1

    def _drain_and_barrier(self, tick_clock, wait_clock):
        nc = self.nc
        drain_inst = nc.sync.drain()
        wait_clock.add_sem_waits(
            drain_inst.ins, ScopedClock({None: tick_clock.global_clock})
        )
        si = drain_inst.ins.sync_info
        if si is not None and len(si.on_wait) > self.MAXW:
            waits = list(si.on_wait)
            drain_inst.ins.sync_info = bass_rust.SyncInfo(
                on_wait=waits[: self.MAXW], on_update=list(si.on_update)
            )
            for i in range(self.MAXW, len(waits), self.MAXW):
                extra = nc.sync.drain()
                extra.ins.sync_info = bass_rust.SyncInfo(
                    on_wait=waits[i : i + self.MAXW], on_update=[]
                )
        nc.all_engine_barrier()
        assert self.sems is not None
        popped = nc._tile_sem_poison_stack.pop()
        assert popped is self._sem_poison
        nc.clear_and_free_semaphores(list(self.sems.allocated().values()))
        nc.all_engine_barrier()


def _split_waits(nc, maxw=1):
    """b16 walrus allows only `maxw` sync-wait commands per instruction.
    Move excess waits onto NOP carriers inserted immediately before the
    instruction in block order (same engine stream → same semantics)."""

    def carrier(engine):
        bi = nc.engines[engine].nop(nofuse=True)
        ins = bi.ins
        for bb in nc.main_func.blocks:
            lst = bb.instructions
            if lst and lst[-1] is ins:
                lst.pop()
                break
        return ins

    for bb in nc.main_func.blocks:
        lst = bb.instructions
        new = []
        for ins in lst:
            si = ins.sync_info
            if si is not None and len(si.on_wait) > maxw:
                waits = list(si.on_wait)
                keep = waits[-maxw:]
                extra = waits[:-maxw]
                for k in range(0, len(extra), maxw):
                    nop = carrier(ins.engine)
                    nop.sync_info = bass_rust.SyncInfo(
                        on_wait=extra[k : k + maxw], on_update=[]
                    )
                    new.append(nop)
                ins.sync_info = bass_rust.SyncInfo(
                    on_wait=keep, on_update=list(si.on_update)
                )
            new.append(ins)
        lst[:] = new


def _build_program():
    nc = bass.Bass()
    dec = nc.declare_dram_parameter("dec", [NB, N, H], F32, isOutput=False)
    sen = nc.declare_dram_parameter("sen", [NB, N, H], F32, isOutput=False)
    Wq = nc.declare_dram_parameter("Wq", [H, H], F32, isOutput=False)
    Wk = nc.declare_dram_parameter("Wk", [H, H], F32, isOutput=False)
    bq = nc.declare_dram_parameter("bq", [H], F32, isOutput=False)
    bk = nc.declare_dram_parameter("bk", [H], F32, isOutput=False)
    wt = nc.declare_dram_parameter("wt", [H], F32, isOutput=False)
    rowmask = nc.declare_dram_parameter("rowmask", [NB, N, N], F32, isOutput=False)
    colmaskT = nc.declare_dram_parameter("colmaskT", [NB, N, N], F32, isOutput=False)
    onehot = nc.declare_dram_parameter("onehot", [NB, N, N], F32, isOutput=False)
    out = nc.declare_dram_parameter("out", [NB, 5, N], F32, isOutput=True)

    from contextlib import ExitStack

    with SafeTileContext(nc) as tc, ExitStack() as ctx:
        consts = ctx.enter_context(tc.tile_pool(name="consts", bufs=1))
        wstage = ctx.enter_context(tc.tile_pool(name="wstage", bufs=1))
        xstage = ctx.enter_context(tc.tile_pool(name="xstage", bufs=2))
        xt_pool = ctx.enter_context(tc.tile_pool(name="xt", bufs=2))
        qk_pool = ctx.enter_context(tc.tile_pool(name="qk", bufs=2))
        epool = ctx.enter_context(tc.tile_pool(name="eraw", bufs=3))
        tpool = ctx.enter_context(tc.tile_pool(name="etanh", bufs=3))
        spool = ctx.enter_context(tc.tile_pool(name="scores", bufs=2))
        mpool = ctx.enter_context(tc.tile_pool(name="masks", bufs=2))
        vpool = ctx.enter_context(tc.tile_pool(name="vecs", bufs=4))
        ps_tr = ctx.enter_context(tc.tile_pool(name="ps_tr", bufs=2, space="PSUM"))
        ps_proj = ctx.enter_context(tc.tile_pool(name="ps_proj", bufs=2, space="PSUM"))
        ps_mv = ctx.enter_context(tc.tile_pool(name="ps_mv", bufs=2, space="PSUM"))

        ident = consts.tile([128, 128], F32)
        make_identity(nc, ident)

        # --- load + cast shared weights -------------------------------
        # Wq/Wk laid out [p, kc, m] so W_bf[:, kc, mc*128:(mc+1)*128] is the
        # [K=128, M=128] stationary tile of W[kc*128:(kc+1)*128, mc*128:...].
        Wq_bf = consts.tile([128, HC, H], BF16, tag="wbf_q")
        Wk_bf = consts.tile([128, HC, H], BF16, tag="wbf_k")
        for W_par, W_bf in ((Wq, Wq_bf), (Wk, Wk_bf)):
            st = wstage.tile([128, HC, H], F32, tag="wstage")
            nc.sync.dma_start(st[:], W_par.rearrange("(a p) m -> p a m", p=128))
            nc.vector.tensor_copy(W_bf[:], st[:])
        bq_sb = consts.tile([128, HC], F32, tag="bq")
        bk_sb = consts.tile([128, HC], F32, tag="bk")
        nc.sync.dma_start(bq_sb[:], bq.rearrange("(a p) -> p a", p=128))
        nc.sync.dma_start(bk_sb[:], bk.rearrange("(a p) -> p a", p=128))
        wt_f = consts.tile([128, HC], F32, tag="wtf")
        nc.sync.dma_start(wt_f[:], wt.rearrange("(a p) -> p a", p=128))
        wt_bf = consts.tile([128, HC], BF16, tag="wtb")
        nc.vector.tensor_copy(wt_bf[:], wt_f[:])

        for b in range(NB):
            # --- load + transpose dec/sen to [h, t] bf16 --------------
            decT = xt_pool.tile([128, HC, N], BF16, tag="decT")
            senT = xt_pool.tile([128, HC, N], BF16, tag="senT")
            for x_par, xT in ((dec, decT), (sen, senT)):
                xs = xstage.tile([128, H], F32, tag="xs")
                nc.sync.dma_start(xs[:], x_par[b])
                for hc in range(HC):
                    ps = ps_tr.tile([128, 128], F32, tag="tr")
                    nc.tensor.transpose(ps[:], xs[:, hc * 128 : (hc + 1) * 128], ident[:])
                    nc.vector.tensor_copy(xT[:, hc, :], ps[:])

            # --- projections q^T = (dec@Wq+bq)^T, k^T = (sen@Wk+bk)^T -
            qT = qk_pool.tile([128, HC, N], F32, tag="qT")
            kT = qk_pool.tile([128, HC, N], BF16, tag="kT")
            for W_bf, xT, b_sb, oT in (
                (Wq_bf, decT, bq_sb, qT),
                (Wk_bf, senT, bk_sb, kT),
            ):
                for mc in range(HC):
                    pp = ps_proj.tile([128, 128], F32, tag="proj")
                    for kc in range(HC):
                        nc.tensor.matmul(
                            pp[:],
                            W_bf[:, kc, mc * 128 : (mc + 1) * 128],
                            xT[:, kc, :],
                            start=(kc == 0),
                            stop=(kc == HC - 1),
                        )
                    nc.vector.tensor_scalar(
                        out=oT[:, mc, :],
                        in0=pp[:],
                        scalar1=b_sb[:, mc : mc + 1],
                        scalar2=None,
                        op0=mybir.AluOpType.add,
                    )

            # --- scores^T[j, t] = sum_h wt[h] tanh(q[t,h]+k[j,h]) -----
            scoresT = spool.tile([128, N], F32, tag="scoresT")
            for tg in range(NTG):
                pmv = ps_mv.tile([128, TG], F32, tag="mv")
                for kc in range(HC):
                    eraw = epool.tile([128, TG, 128], BF16, tag="eraw")
                    for ti in range(TG):
                        t = tg * TG + ti
                        nc.vector.tensor_scalar(
                            out=eraw[:, ti, :],
                            in0=kT[:, kc, :],
                            scalar1=qT[:, kc, t : t + 1],
                            scalar2=None,
                            op0=mybir.AluOpType.add,
                        )
                    etanh = tpool.tile([128, TG, 128], BF16, tag="etanh")
                    nc.scalar.activation(
                        etanh[:], eraw[:], mybir.ActivationFunctionType.Tanh
                    )
                    for ti in range(TG):
                        nc.tensor.matmul(
                            pmv[:, ti : ti + 1],
                            etanh[:, ti, :],
                            wt_bf[:, kc : kc + 1],
                            start=(kc == 0),
                            stop=(kc == HC - 1),
                        )
                nc.vector.tensor_copy(scoresT[:, tg * TG : (tg + 1) * TG], pmv[:])

            # --- col softmax stats (over t; scoresT layout [j, t]) ----
            cmT = mpool.tile([128, N], F32, tag="cmT")
            nc.sync.dma_start(cmT[:], colmaskT[b])
            cmask = spool.tile([128, N], F32, tag="cmask")
            nc.vector.tensor_tensor(
                out=cmask[:], in0=scoresT[:], in1=cmT[:], op=mybir.AluOpType.add
            )
            negm2 = vpool.tile([128, 1], F32, tag="negm2")
            nc.vector.tensor_reduce(
                out=negm2[:], in_=cmask[:], axis=mybir.AxisListType.X,
                op=mybir.AluOpType.max, negate=True,
            )
            esc2 = spool.tile([128, N], BF16, tag="esc2")
            s2 = vpool.tile([128, 1], F32, tag="s2")
            nc.scalar.activation(
                esc2[:], cmask[:], mybir.ActivationFunctionType.Exp,
                bias=negm2[:], scale=1.0, accum_out=s2[:],
            )

            # --- transpose scores to [t, j] ---------------------------
            scoresR = spool.tile([128, N], F32, tag="scoresR")
            pst = ps_tr.tile([128, 128], F32, tag="tr")
            nc.tensor.transpose(pst[:], scoresT[:], ident[:])
            nc.vector.tensor_copy(scoresR[:], pst[:])

            # --- row softmax stats (over j) ---------------------------
            rm = mpool.tile([128, N], F32, tag="rm")
            nc.sync.dma_start(rm[:], rowmask[b])
            rmask = spool.tile([128, N], F32, tag="rmask")
            nc.vector.tensor_tensor(
                out=rmask[:], in0=scoresR[:], in1=rm[:], op=mybir.AluOpType.add
            )
            negm1 = vpool.tile([128, 1], F32, tag="negm1")
            nc.vector.tensor_reduce(
                out=negm1[:], in_=rmask[:], axis=mybir.AxisListType.X,
                op=mybir.AluOpType.max, negate=True,
            )
            esc1 = spool.tile([128, N], BF16, tag="esc1")
            s1 = vpool.tile([128, 1], F32, tag="s1")
            nc.scalar.activation(
                esc1[:], rmask[:], mybir.ActivationFunctionType.Exp,
                bias=negm1[:], scale=1.0, accum_out=s1[:],
            )

            # --- gather raw scores at target via one-hot --------------
            oh = mpool.tile([128, N], F32, tag="oh")
            nc.sync.dma_start(oh[:], onehot[b])
            gm = spool.tile([128, N], F32, tag="gm")
            nc.vector.tensor_tensor(
                out=gm[:], in0=scoresR[:], in1=oh[:], op=mybir.AluOpType.mult
            )
            gsc = vpool.tile([128, 1], F32, tag="gsc")
            nc.vector.tensor_reduce(
                out=gsc[:], in_=gm[:], axis=mybir.AxisListType.X,
                op=mybir.AluOpType.add,
            )

            # --- ship stats -------------------------------------------
            for i, v in enumerate((negm1, s1, negm2, s2, gsc)):
                nc.sync.dma_start(out[b, i, :], v[:])

    _split_waits(nc, maxw=1)
    return nc


_CACHE = {}


def _get_program():
    if "nc" not in _CACHE:
        _CACHE["nc"] = _build_program()
    return _CACHE["nc"]


def host_prep(dec_outputs, sen_vec, Wq, bq, Wk, bk, wt, bt, target, tgt_len):
    """Shard inputs + build index-derived masks. Returns (in_maps, aux)."""
    dec_outputs = np.ascontiguousarray(dec_outputs, dtype=np.float32)
    sen_vec = np.ascontiguousarray(sen_vec, dtype=np.float32)
    Wq = np.ascontiguousarray(Wq, dtype=np.float32)
    bq = np.ascontiguousarray(bq, dtype=np.float32)
    Wk = np.ascontiguousarray(Wk, dtype=np.float32)
    bk = np.ascontiguousarray(bk, dtype=np.float32)
    wt = np.ascontiguousarray(wt, dtype=np.float32)
    bt = np.ascontiguousarray(bt, dtype=np.float32)
    target = np.ascontiguousarray(target, dtype=np.int32)
    tgt_len = np.ascontiguousarray(tgt_len, dtype=np.int32)

    ar = np.arange(N)
    oh = (target[..., None] == ar[None, None, :]).astype(np.float32)  # [B,t,j]
    cum = np.cumsum(oh, axis=1)
    pointed = np.concatenate([np.zeros_like(cum[:, :1]), cum[:, :-1]], axis=1) > 0
    validj = ar[None, :] < tgt_len[:, None]                            # [B,N]
    row_m = np.where(pointed | ~validj[:, None, :], NEG, np.float32(0)).astype(
        np.float32
    )
    col_m = np.where(
        ~(validj[:, None, :] & validj[:, :, None]), NEG, np.float32(0)
    ).astype(np.float32)
    col_mT = np.ascontiguousarray(col_m.transpose(0, 2, 1))            # [B,j,t]

    in_maps = []
    for c in range(NCORES):
        sl = slice(c * NB, (c + 1) * NB)
        in_maps.append(
            dict(
                dec=np.ascontiguousarray(dec_outputs[sl]),
                sen=np.ascontiguousarray(sen_vec[sl]),
                Wq=Wq, Wk=Wk, bq=bq, bk=bk, wt=wt,
                rowmask=np.ascontiguousarray(row_m[sl]),
                colmaskT=np.ascontiguousarray(col_mT[sl]),
                onehot=np.ascontiguousarray(oh[sl]),
            )
        )
    aux = dict(
        row_m=row_m, col_m=col_m, validj=validj, target=target,
        tgt_len=tgt_len, bt=bt,
    )
    return in_maps, aux


def host_combine(stats, aux):
    """Combine per-batch device stats [B,5,N] into the scalar loss (fp32)."""
    negm1, s1, negm2, s2, gsc = (stats[:, i, :] for i in range(5))
    target = aux["target"]
    bt0 = np.float32(aux["bt"][0])
    lse_row = (-negm1 + np.log(s1) + bt0).astype(np.float32)           # [B,t]
    lse_col = (-negm2 + np.log(s2) + bt0).astype(np.float32)           # [B,j]

    bi = np.arange(B)[:, None]
    g_bt = (gsc + bt0).astype(np.float32)                              # [B,t]
    ti = np.arange(N)[None, :]
    row_m_at = aux["row_m"][bi, ti, target]                            # [B,t]
    col_m_at = aux["col_m"][bi, ti, target]                            # [B,t]
    e_row_at = np.where(row_m_at == 0, g_bt, NEG).astype(np.float32)
    e_col_at = np.where(col_m_at == 0, g_bt, NEG).astype(np.float32)
    lse_col_at = lse_col[bi, target].astype(np.float32)                # [B,t]

    validt = aux["validj"]                                             # square mask
    nll = np.where(validt, lse_row - e_row_at, np.float32(0)).astype(np.float32)
    nll2 = np.where(validt, lse_col_at - e_col_at, np.float32(0)).astype(np.float32)

    lens = aux["tgt_len"].astype(np.float32)
    d1 = (lens + np.float32(1e-20) - np.float32(1.0)).astype(np.float32)
    row_loss = np.float32(np.mean((nll.sum(axis=1) / d1).astype(np.float32)))
    col_loss = np.float32(np.mean((nll2.sum(axis=1) / (lens * d1)).astype(np.float32)))
    return np.asarray(row_loss + col_loss, dtype=np.float32)


def kernel(dec_outputs, sen_vec, Wq, bq, Wk, bk, wt, bt, target, tgt_len):
    in_maps, aux = host_prep(
        dec_outputs, sen_vec, Wq, bq, Wk, bk, wt, bt, target, tgt_len
    )
    nc = _get_program()
    res = run_bass_kernel_spmd(nc, in_maps, core_ids=list(range(NCORES)))
    stats = np.concatenate([res.results[c]["out"] for c in range(NCORES)], axis=0)
    return host_combine(stats, aux)
